# revision 1
# baseline (speedup 1.0000x reference)
"""Trainium2 Bass kernel for masked-LSTM-last + dense readout.

Reference semantics (B=256, T=4096, F=32, H=128):
    h_b = LSTM(inputs[b, :seq_lens[b]])   (Keras-style masked LSTM, final h)
    out[b] = h_b @ Wf + bf

Strategy:
  - Pure data-parallel: 32 samples per core across 8 cores.
  - Recurrence runs in "transposed" layout: z_t is [4H gate-units, Bl samples],
    kept as 4 PSUM banks (one per gate, chunked 16 steps per bank).
  - x @ W + b for a 16-step chunk is 4 matmuls (N=512) with stationary
    W~ = [W; b] (33 rows: 32 features + constant-1 row).
  - Per step, 4 small matmuls (stationary U_g [128,128], N=32) accumulate
    h_{t-1} @ U into the same banks.
  - Gates: sigmoid over the i,f,o banks in ONE ACT instruction (bank-strided
    access pattern), tanh over the g bank; cell update on the vector engine.
  - The recurrence is run UNMASKED; h_t history is streamed to DRAM (fp16) and
    the host picks h at t = seq_len-1 (identical to the masked-freeze result)
    and applies the final dense layer.
"""

import os
import sys

import numpy as np

if "/opt/trn_rl_repo" not in sys.path:
    sys.path.insert(0, "/opt/trn_rl_repo")

F = 32
H = 128
NCORES = 8
CH = 16  # steps per PSUM z-chunk (one bank per gate)
XCH = 64  # steps per x-input DMA chunk

# gate order in the PSUM banks: i, f, o, g  (so sigmoid covers banks 0..2)
GATE_PERM = np.r_[0:256, 384:512, 256:384]

_NC_CACHE = {}

KMODE = os.environ.get("KMODE", "v5")


def _register_lstm_fma():
    """Runtime-register the custom DVE op  out = (in0*in1 + in1) * s0.

    Used for the one-func-tanh LSTM cell:  with t* = tanh(z*/2) the gates are
    sigma(z) = (t+1)/2, so  f*c = (tf*c + c)*0.5  etc.
    """
    import numpy as _np
    from concourse import dve_ops
    from concourse.dve_spec import C0, Spec, Src0, Src1, lower, _has_src1
    from concourse.dve_table_gen import dve_ver_for  # noqa: F401
    from concourse.dve_uop import DveOpSpec

    for op in dve_ops.OPS:
        if op.name == "LSTM_FMA_ANT":
            return op
    op = dve_ops.DveOp(
        "LSTM_FMA_ANT",
        Spec(
            body=(Src0 * Src1 + Src1) * C0,
            reference=lambda in0, in1, s0, s1, imm2: (
                in0.astype(_np.float32) * in1 + in1
            )
            * s0,
        ),
        subdim=False,
        uops_sha={},
    )
    dve_ops.OPS.append(op)
    dve_ops.CUSTOM_DVE_SPECS[op.name] = op.spec
    row = dve_ops._CUSTOM_DVE_ROW_BASE + len(dve_ops.OPS) - 1
    assert row < 0x20
    dve_ops._SUB_OPCODE_FOR_NAME[op.name] = row
    for ver in ("v3",):
        compiled = DveOpSpec(
            name=op.name,
            opcode=row,
            uops=lower(op.spec, ver=ver),
            rd1_en=_has_src1(op.spec),
        )
        dve_ops._COMPILE_CACHE[(op.name, ver)] = compiled
    return op


def _build_nc_v2(T, repeat=1):
    """One-func-tanh cell + fused DVE ops + 2-group stagger."""
    import concourse.mybir as mybir
    import concourse.tile as tile
    from concourse import bacc

    f16 = mybir.dt.float16
    f32 = mybir.dt.float32
    AF = mybir.ActivationFunctionType
    fma = _register_lstm_fma()

    nch = T // CH
    assert T % XCH == 0 and XCH % CH == 0
    BL = 16  # samples per group

    nc = bacc.Bacc("TRN2", num_devices=NCORES)
    xt_d = nc.dram_tensor("xt", [F + 1, T, 32], f16, kind="ExternalInput").ap()
    u_d = nc.dram_tensor("u", [H, 4 * H], f16, kind="ExternalInput").ap()
    wt_d = nc.dram_tensor("wt", [F + 1, 4 * H], f16, kind="ExternalInput").ap()
    hist_d = nc.dram_tensor("hist", [nch, H, CH * 32], f16, kind="ExternalOutput").ap()

    with tile.TileContext(nc) as tc:
        with (
            tc.tile_pool(name="const", bufs=1) as constp,
            tc.tile_pool(name="xin", bufs=2) as xp,
            tc.tile_pool(name="zp", bufs=2, space="PSUM") as zp,
            tc.tile_pool(name="gp", bufs=4) as gp,
            tc.tile_pool(name="tp", bufs=4) as tp,
            tc.tile_pool(name="cp", bufs=4) as cp,
            tc.tile_pool(name="hp", bufs=3) as hp,
        ):
            u_sb = constp.tile([H, 4 * H], f16, tag="u")
            nc.sync.dma_start(u_sb[:], u_d)
            wt_sb = constp.tile([F + 1, 4 * H], f16, tag="wt")
            nc.sync.dma_start(wt_sb[:], wt_d)
            h0 = constp.tile([H, 32], f16, tag="h0")
            nc.vector.memset(h0[:], 0.0)
            c0 = constp.tile([H, 32], f32, tag="c0")
            nc.vector.memset(c0[:], 0.0)

            prev_h = [h0[:, 0:BL], h0[:, BL:32]]
            c_cur = [c0[:, 0:BL], c0[:, BL:32]]
            xt_sb = None
            for c in [cc for _ in range(repeat) for cc in range(nch)]:
                if (c * CH) % XCH == 0:
                    xt_sb = xp.tile([F + 1, XCH * 32], f16, tag="x")
                    t0 = c * CH
                    nc.sync.dma_start(xt_sb[:], xt_d[:, t0 : t0 + XCH, :])
                xoff = ((c * CH) % XCH) * 32

                z = zp.tile([H, 4 * 512], f32, tag="z")
                hist_sb = hp.tile([H, CH * 32], f16, tag="h")

                for g in range(4):
                    nc.tensor.matmul(
                        z[:, g * 512 : (g + 1) * 512],
                        wt_sb[:, g * H : (g + 1) * H],
                        xt_sb[:, xoff : xoff + 512],
                        start=True,
                        stop=False,
                        skip_group_check=True,
                    )

                for s in range(CH):
                    for grp in range(2):
                        off = s * 32 + grp * BL
                        for g in range(4):
                            nc.tensor.matmul(
                                z[:, g * 512 + off : g * 512 + off + BL],
                                u_sb[:, g * H : (g + 1) * H],
                                prev_h[grp],
                                start=False,
                                stop=True,
                                skip_group_check=True,
                            )
                        gates = gp.tile([H, 4 * BL], f16, tag=f"g{grp}")
                        zs = z.rearrange("p (g n) -> p g n", g=4)[
                            :, :, off : off + BL
                        ]
                        gs = gates.rearrange("p (g n) -> p g n", g=4)
                        nc.scalar.activation(gs, zs, AF.Tanh)
                        ti = gates[:, 0:BL]
                        tf = gates[:, BL : 2 * BL]
                        tg = gates[:, 2 * BL : 3 * BL]
                        to = gates[:, 3 * BL : 4 * BL]
                        p2 = tp.tile([H, BL], f32, tag=f"p{grp}")
                        nc.vector._custom_dve(fma, out=p2[:], in0=ti, in1=tg, s0=0.5)
                        q2 = tp.tile([H, BL], f32, tag=f"q{grp}")
                        nc.vector._custom_dve(
                            fma, out=q2[:], in0=tf, in1=c_cur[grp], s0=0.5
                        )
                        c_new = cp.tile([H, BL], f32, tag=f"c{grp}")
                        nc.vector.tensor_add(c_new[:], p2[:], q2[:])
                        tc_t = tp.tile([H, BL], f16, tag=f"t{grp}")
                        nc.scalar.activation(tc_t[:], c_new[:], AF.Tanh)
                        nc.vector._custom_dve(
                            fma,
                            out=hist_sb[:, off : off + BL],
                            in0=to,
                            in1=tc_t[:],
                            s0=0.5,
                        )
                        prev_h[grp] = hist_sb[:, off : off + BL]
                        c_cur[grp] = c_new[:]

                nc.sync.dma_start(hist_d[c], hist_sb[:])

    if not nc.is_finalized():
        nc.finalize()
    return nc


def _build_nc(T, repeat=1, mode="full"):
    import concourse.mybir as mybir
    import concourse.tile as tile
    from concourse import bacc

    f16 = mybir.dt.float16
    f32 = mybir.dt.float32
    AF = mybir.ActivationFunctionType

    nch = T // CH
    assert T % XCH == 0 and XCH % CH == 0

    nc = bacc.Bacc("TRN2", num_devices=NCORES)
    xt_d = nc.dram_tensor("xt", [F + 1, T, 32], f16, kind="ExternalInput").ap()
    u_d = nc.dram_tensor("u", [H, 4 * H], f16, kind="ExternalInput").ap()
    wt_d = nc.dram_tensor("wt", [F + 1, 4 * H], f16, kind="ExternalInput").ap()
    hist_d = nc.dram_tensor("hist", [nch, H, CH * 32], f16, kind="ExternalOutput").ap()

    with tile.TileContext(nc) as tc:
        with (
            tc.tile_pool(name="const", bufs=1) as constp,
            tc.tile_pool(name="xin", bufs=2) as xp,
            tc.tile_pool(name="zp", bufs=2, space="PSUM") as zp,
            tc.tile_pool(name="gp", bufs=3) as gp,
            tc.tile_pool(name="tp", bufs=3) as tp,
            tc.tile_pool(name="cp", bufs=3) as cp,
            tc.tile_pool(name="hp", bufs=3) as hp,
        ):
            u_sb = constp.tile([H, 4 * H], f16, tag="u")
            nc.sync.dma_start(u_sb[:], u_d)
            wt_sb = constp.tile([F + 1, 4 * H], f16, tag="wt")
            nc.sync.dma_start(wt_sb[:], wt_d)
            h0 = constp.tile([H, 32], f16, tag="h0")
            nc.vector.memset(h0[:], 0.0)
            c0 = constp.tile([H, 32], f32, tag="c0")
            nc.vector.memset(c0[:], 0.0)

            prev_h = h0[:]
            c_cur = c0[:]
            xt_sb = None
            for c in [cc for _ in range(repeat) for cc in range(nch)]:
                if (c * CH) % XCH == 0:
                    xt_sb = xp.tile([F + 1, XCH * 32], f16, tag="x")
                    t0 = c * CH
                    nc.sync.dma_start(xt_sb[:], xt_d[:, t0 : t0 + XCH, :])
                xoff = ((c * CH) % XCH) * 32

                z = zp.tile([H, 4 * 512], f32, tag="z")
                hist_sb = hp.tile([H, CH * 32], f16, tag="h")

                # x @ W~ for the whole chunk: one matmul per gate into its bank
                for g in range(4):
                    nc.tensor.matmul(
                        z[:, g * 512 : (g + 1) * 512],
                        wt_sb[:, g * H : (g + 1) * H],
                        xt_sb[:, xoff : xoff + 512],
                        start=True,
                        stop=False,
                        skip_group_check=True,
                    )

                for s in range(CH):
                    # h_{t-1} @ U accumulated into the 4 banks at this step's cols
                    if True:
                        for g in range(4):
                            nc.tensor.matmul(
                                z[:, g * 512 + s * 32 : g * 512 + (s + 1) * 32],
                                u_sb[:, g * H : (g + 1) * H],
                                prev_h,
                                start=False,
                                stop=True,
                                skip_group_check=True,
                            )
                    if mode == "pe":
                        continue
                    gates = gp.tile([H, 128], f16, tag="g")
                    z3 = z[:, 0:1536].rearrange("p (g n) -> p g n", g=3)[
                        :, :, s * 32 : (s + 1) * 32
                    ]
                    g3 = gates[:, 0:96].rearrange("p (g n) -> p g n", g=3)
                    nc.scalar.activation(g3, z3, AF.Sigmoid)
                    nc.scalar.activation(
                        gates[:, 96:128],
                        z[:, 1536 + s * 32 : 1536 + (s + 1) * 32],
                        AF.Tanh,
                    )
                    if mode == "pe_act":
                        prev_h = gates[:, 0:32]
                        continue
                    p = tp.tile([H, 32], f16, tag="p")
                    nc.vector.tensor_mul(p[:], gates[:, 0:32], gates[:, 96:128])
                    q = tp.tile([H, 32], f32, tag="q")
                    nc.vector.tensor_mul(q[:], gates[:, 32:64], c_cur)
                    c_new = cp.tile([H, 32], f32, tag="c")
                    nc.vector.tensor_add(c_new[:], p[:], q[:])
                    tc_t = tp.tile([H, 32], f16, tag="tc")
                    nc.scalar.activation(tc_t[:], c_new[:], AF.Tanh)
                    nc.vector.tensor_mul(
                        hist_sb[:, s * 32 : (s + 1) * 32], gates[:, 64:96], tc_t[:]
                    )
                    prev_h = hist_sb[:, s * 32 : (s + 1) * 32]
                    c_cur = c_new[:]

                if mode == "full":
                    nc.sync.dma_start(hist_d[c], hist_sb[:])

    if not nc.is_finalized():
        nc.finalize()
    return nc


def _build_nc_v3(T, repeat=1):
    """v2 + per-group PSUM banks so the two sample-group pipelines decouple.

    Group g (16 samples) owns 4 PSUM banks; a z-chunk covers 32 steps
    (512 cols = 32 steps x 16 samples per gate-bank).
    """
    import concourse.mybir as mybir
    import concourse.tile as tile
    from concourse import bacc

    f16 = mybir.dt.float16
    f32 = mybir.dt.float32
    AF = mybir.ActivationFunctionType
    fma = _register_lstm_fma()

    CH2 = 32  # steps per z-chunk per group
    assert T % CH2 == 0 and T % XCH == 0 and XCH % CH2 == 0
    nch = T // CH
    BL = 16

    nc = bacc.Bacc("TRN2", num_devices=NCORES)
    xt_d = nc.dram_tensor("xt", [F + 1, T, 32], f16, kind="ExternalInput").ap()
    u_d = nc.dram_tensor("u", [H, 4 * H], f16, kind="ExternalInput").ap()
    wt_d = nc.dram_tensor("wt", [F + 1, 4 * H], f16, kind="ExternalInput").ap()
    hist_d = nc.dram_tensor("hist", [nch, H, CH * 32], f16, kind="ExternalOutput").ap()

    with tile.TileContext(nc) as tc:
        with (
            tc.tile_pool(name="const", bufs=1) as constp,
            tc.tile_pool(name="xin", bufs=2) as xp,
            tc.tile_pool(name="zpa", bufs=1, space="PSUM") as zpa,
            tc.tile_pool(name="zpb", bufs=1, space="PSUM") as zpb,
            tc.tile_pool(name="gp", bufs=6) as gp,
            tc.tile_pool(name="tp", bufs=6) as tp,
            tc.tile_pool(name="cp", bufs=6) as cp,
            tc.tile_pool(name="hp", bufs=4) as hp,
        ):
            u_sb = constp.tile([H, 4 * H], f16, tag="u")
            nc.sync.dma_start(u_sb[:], u_d)
            wt_sb = constp.tile([F + 1, 4 * H], f16, tag="wt")
            nc.sync.dma_start(wt_sb[:], wt_d)
            h0 = constp.tile([H, 32], f16, tag="h0")
            nc.vector.memset(h0[:], 0.0)
            c0 = constp.tile([H, 32], f32, tag="c0")
            nc.vector.memset(c0[:], 0.0)

            zpools = [zpa, zpb]
            prev_h = [h0[:, 0:BL], h0[:, BL:32]]
            c_cur = [c0[:, 0:BL], c0[:, BL:32]]
            z_grp = [None, None]
            xt_sb = None
            hist_sb = None
            total = T * repeat
            for t_abs in range(total):
                t = t_abs % T
                if t % XCH == 0:
                    xt_sb = xp.tile([F + 1, XCH * 32], f16, tag="x")
                    nc.sync.dma_start(xt_sb[:], xt_d[:, t : t + XCH, :])
                if t % CH == 0:
                    if hist_sb is not None:
                        nc.sync.dma_start(hist_d[(t_abs // CH - 1) % nch], hist_sb[:])
                    hist_sb = hp.tile([H, CH * 32], f16, tag="h")
                for grp in range(2):
                    if t % CH2 == 0:
                        z_grp[grp] = zpools[grp].tile(
                            [H, 4 * 512], f32, tag="z", name=f"z{grp}"
                        )
                        xoff = (t % XCH) * 32 + grp * BL
                        for g in range(4):
                            nc.tensor.matmul(
                                z_grp[grp][:, g * 512 : (g + 1) * 512],
                                wt_sb[:, g * H : (g + 1) * H],
                                xt_sb.rearrange("p (t n) -> p t n", n=32)[
                                    :, (t % XCH) : (t % XCH) + CH2, grp * BL : (grp + 1) * BL
                                ],
                                start=True,
                                stop=False,
                                skip_group_check=True,
                            )
                    z = z_grp[grp]
                    s2 = t % CH2
                    off = s2 * BL
                    for g in range(4):
                        nc.tensor.matmul(
                            z[:, g * 512 + off : g * 512 + off + BL],
                            u_sb[:, g * H : (g + 1) * H],
                            prev_h[grp],
                            start=False,
                            stop=True,
                            skip_group_check=True,
                        )
                    gates = gp.tile([H, 4 * BL], f16, tag=f"g{grp}")
                    zs = z.rearrange("p (g n) -> p g n", g=4)[:, :, off : off + BL]
                    gs = gates.rearrange("p (g n) -> p g n", g=4)
                    nc.scalar.activation(gs, zs, AF.Tanh)
                    ti = gates[:, 0:BL]
                    tf = gates[:, BL : 2 * BL]
                    tg = gates[:, 2 * BL : 3 * BL]
                    to = gates[:, 3 * BL : 4 * BL]
                    p2 = tp.tile([H, BL], f32, tag=f"p{grp}")
                    nc.vector._custom_dve(fma, out=p2[:], in0=ti, in1=tg, s0=0.5)
                    q2 = tp.tile([H, BL], f32, tag=f"q{grp}")
                    nc.vector._custom_dve(fma, out=q2[:], in0=tf, in1=c_cur[grp], s0=0.5)
                    c_new = cp.tile([H, BL], f32, tag=f"c{grp}")
                    nc.vector.tensor_add(c_new[:], p2[:], q2[:])
                    tc_t = tp.tile([H, BL], f16, tag=f"t{grp}")
                    nc.scalar.activation(tc_t[:], c_new[:], AF.Tanh)
                    hoff = (t % CH) * 32 + grp * BL
                    nc.vector._custom_dve(
                        fma,
                        out=hist_sb[:, hoff : hoff + BL],
                        in0=to,
                        in1=tc_t[:],
                        s0=0.5,
                    )
                    prev_h[grp] = hist_sb[:, hoff : hoff + BL]
                    c_cur[grp] = c_new[:]
            nc.sync.dma_start(hist_d[(total // CH - 1) % nch], hist_sb[:])

    if not nc.is_finalized():
        nc.finalize()
    return nc


def _build_nc_v4(T, repeat=1):
    """v3 + fused PQ custom-DVE op via interleaved access patterns.

    Per (group, step) a G-tile [128, 96] fp32 holds
    [ti | tf | tg | to | c_prev | tc]; one LSTM_FMA instruction computes the
    interleaved pair (p2, q2) = ((ti*tg+tg)*0.5, (tf*c+c)*0.5), a strided add
    produces c' straight into the next G-tile, tanh(c') lands back in this
    G-tile, and one more LSTM_FMA produces h.
    """
    import concourse.mybir as mybir
    import concourse.tile as tile
    from concourse import bacc

    f16 = mybir.dt.float16
    f32 = mybir.dt.float32
    AF = mybir.ActivationFunctionType
    fma = _register_lstm_fma()

    CH2 = 32
    assert T % CH2 == 0 and T % XCH == 0 and XCH % CH2 == 0
    nch = T // CH
    BL = 16

    nc = bacc.Bacc("TRN2", num_devices=NCORES)
    xt_d = nc.dram_tensor("xt", [F + 1, T, 32], f16, kind="ExternalInput").ap()
    u_d = nc.dram_tensor("u", [H, 4 * H], f16, kind="ExternalInput").ap()
    wt_d = nc.dram_tensor("wt", [F + 1, 4 * H], f16, kind="ExternalInput").ap()
    hist_d = nc.dram_tensor("hist", [nch, H, CH * 32], f16, kind="ExternalOutput").ap()

    with tile.TileContext(nc) as tc:
        with (
            tc.tile_pool(name="const", bufs=1) as constp,
            tc.tile_pool(name="xin", bufs=2) as xp,
            tc.tile_pool(name="zpa", bufs=1, space="PSUM") as zpa,
            tc.tile_pool(name="zpb", bufs=1, space="PSUM") as zpb,
            tc.tile_pool(name="gp", bufs=6) as gp,
            tc.tile_pool(name="tp", bufs=6) as tp,
            tc.tile_pool(name="hp", bufs=4) as hp,
        ):
            u_sb = constp.tile([H, 4 * H], f16, tag="u")
            nc.sync.dma_start(u_sb[:], u_d)
            wt_sb = constp.tile([F + 1, 4 * H], f16, tag="wt")
            nc.sync.dma_start(wt_sb[:], wt_d)
            h0 = constp.tile([H, 32], f16, tag="h0")
            nc.vector.memset(h0[:], 0.0)

            zpools = [zpa, zpb]
            prev_h = [h0[:, 0:BL], h0[:, BL:32]]
            g_cur = []
            for grp in range(2):
                g_t = gp.tile([H, 96], f32, tag=f"g{grp}", name=f"ginit{grp}")
                nc.vector.memset(g_t[:, 64:80], 0.0)  # c_{-1} = 0
                g_cur.append(g_t)
            z_grp = [None, None]
            xt_sb = None
            hist_sb = None
            total = T * repeat
            for t_abs in range(total):
                t = t_abs % T
                if t % XCH == 0:
                    xt_sb = xp.tile([F + 1, XCH * 32], f16, tag="x")
                    nc.sync.dma_start(xt_sb[:], xt_d[:, t : t + XCH, :])
                if t % CH == 0:
                    if hist_sb is not None:
                        nc.sync.dma_start(hist_d[(t_abs // CH - 1) % nch], hist_sb[:])
                    hist_sb = hp.tile([H, CH * 32], f16, tag="h")
                for grp in range(2):
                    if t % CH2 == 0:
                        z_grp[grp] = zpools[grp].tile(
                            [H, 4 * 512], f32, tag="z", name=f"z{grp}"
                        )
                        for g in range(4):
                            nc.tensor.matmul(
                                z_grp[grp][:, g * 512 : (g + 1) * 512],
                                wt_sb[:, g * H : (g + 1) * H],
                                xt_sb.rearrange("p (t n) -> p t n", n=32)[
                                    :,
                                    (t % XCH) : (t % XCH) + CH2,
                                    grp * BL : (grp + 1) * BL,
                                ],
                                start=True,
                                stop=False,
                                skip_group_check=True,
                            )
                    z = z_grp[grp]
                    off = (t % CH2) * BL
                    for g in range(4):
                        nc.tensor.matmul(
                            z[:, g * 512 + off : g * 512 + off + BL],
                            u_sb[:, g * H : (g + 1) * H],
                            prev_h[grp],
                            start=False,
                            stop=True,
                            skip_group_check=True,
                        )
                    gt = g_cur[grp]
                    zs = z.rearrange("p (g n) -> p g n", g=4)[:, :, off : off + BL]
                    nc.scalar.activation(
                        gt[:, 0:64].rearrange("p (g n) -> p g n", g=4), zs, AF.Tanh
                    )
                    g_next = gp.tile([H, 96], f32, tag=f"g{grp}", name=f"gn{grp}")
                    # (p2, q2) interleaved: pairs (ti_k, tf_k) x (tg_k, c_k)
                    pq = tp.tile([H, 2 * BL], f32, tag=f"pq{grp}")
                    nc.vector._custom_dve(
                        fma,
                        out=pq.rearrange("p (k a) -> p k a", a=2),
                        in0=gt[:, 0:32].rearrange("p (a k) -> p k a", a=2),
                        in1=gt[:, 32:96].rearrange("p (a k) -> p k a", a=2)[
                            :, 0:BL, :
                        ],
                        s0=0.5,
                    )
                    # c' = p2 + q2 -> next G-tile's c slot
                    pqv = pq.rearrange("p (k a) -> p a k", a=2)
                    nc.vector.tensor_add(g_next[:, 64:80], pqv[:, 0, :], pqv[:, 1, :])
                    # tc = tanh(c')
                    nc.scalar.activation(gt[:, 80:96], g_next[:, 64:80], AF.Tanh)
                    hoff = (t % CH) * 32 + grp * BL
                    nc.vector._custom_dve(
                        fma,
                        out=hist_sb[:, hoff : hoff + BL],
                        in0=gt[:, 48:64],
                        in1=gt[:, 80:96],
                        s0=0.5,
                    )
                    prev_h[grp] = hist_sb[:, hoff : hoff + BL]
                    g_cur[grp] = g_next
            nc.sync.dma_start(hist_d[(total // CH - 1) % nch], hist_sb[:])

    if not nc.is_finalized():
        nc.finalize()
    return nc


def _build_nc_v5(T, repeat=1):
    """v3 + fused PQ custom-DVE op via interleaved access patterns.

    Per (group, step) a G-tile [128, 96] fp32 holds
    [ti | tf | tg | to | c_prev | tc]; one LSTM_FMA instruction computes the
    interleaved pair (p2, q2) = ((ti*tg+tg)*0.5, (tf*c+c)*0.5), a strided add
    produces c' straight into the next G-tile, tanh(c') lands back in this
    G-tile, and one more LSTM_FMA produces h.
    """
    import concourse.mybir as mybir
    import concourse.tile as tile
    from concourse import bacc

    f16 = mybir.dt.float16
    f32 = mybir.dt.float32
    AF = mybir.ActivationFunctionType
    fma = _register_lstm_fma()

    CH2 = 32
    assert T % CH2 == 0 and T % XCH == 0 and XCH % CH2 == 0
    nch = T // CH
    BL = 16

    nc = bacc.Bacc("TRN2", num_devices=NCORES)
    xt_d = nc.dram_tensor("xt", [F + 1, T, 32], f16, kind="ExternalInput").ap()
    u_d = nc.dram_tensor("u", [H, 4 * H], f16, kind="ExternalInput").ap()
    wt_d = nc.dram_tensor("wt", [F + 1, 4 * H], f16, kind="ExternalInput").ap()
    hist_d = nc.dram_tensor("hist", [nch, H, CH * 32], f16, kind="ExternalOutput").ap()

    with tile.TileContext(nc) as tc:
        with (
            tc.tile_pool(name="const", bufs=1) as constp,
            tc.tile_pool(name="xin", bufs=2) as xp,
            tc.tile_pool(name="zpa", bufs=1, space="PSUM") as zpa,
            tc.tile_pool(name="zpb", bufs=1, space="PSUM") as zpb,
            tc.tile_pool(name="gp", bufs=6) as gp,
            tc.tile_pool(name="tp", bufs=6) as tp,
            tc.tile_pool(name="hp", bufs=4) as hp,
        ):
            u_sb = constp.tile([H, 4 * H], f16, tag="u")
            nc.sync.dma_start(u_sb[:], u_d)
            wt_sb = constp.tile([F + 1, 4 * H], f16, tag="wt")
            nc.sync.dma_start(wt_sb[:], wt_d)
            h0 = constp.tile([H, 32], f16, tag="h0")
            nc.vector.memset(h0[:], 0.0)

            ORDER = int(os.environ.get("V5ORDER", "1"))
            zpools = [zpa, zpb]
            g_nxt = [None, None]
            prev_h = [h0[:, 0:BL], h0[:, BL:32]]
            g_cur = []
            for grp in range(2):
                g_t = gp.tile([H, 96], f32, tag=f"g{grp}", name=f"ginit{grp}")
                nc.vector.memset(g_t[:, 64:80], 0.0)  # c_{-1} = 0
                g_cur.append(g_t)
            z_grp = [None, None]
            xt_sb = None
            hist_sb = None
            total = T * repeat
            for t_abs in range(total):
                t = t_abs % T
                if t % XCH == 0:
                    xt_sb = xp.tile([F + 1, XCH * 32], f16, tag="x")
                    nc.sync.dma_start(xt_sb[:], xt_d[:, t : t + XCH, :])
                if t % CH == 0:
                    if hist_sb is not None:
                        nc.sync.dma_start(hist_d[(t_abs // CH - 1) % nch], hist_sb[:])
                    hist_sb = hp.tile([H, CH * 32], f16, tag="h")
                def stage_mm(grp):
                    if t % CH2 == 0:
                        z_grp[grp] = zpools[grp].tile(
                            [H, 4 * 512], f32, tag="z", name=f"z{grp}"
                        )
                        for g in range(4):
                            nc.tensor.matmul(
                                z_grp[grp][:, g * 512 : (g + 1) * 512],
                                wt_sb[:, g * H : (g + 1) * H],
                                xt_sb.rearrange("p (t n) -> p t n", n=32)[
                                    :,
                                    (t % XCH) : (t % XCH) + CH2,
                                    grp * BL : (grp + 1) * BL,
                                ],
                                start=True,
                                stop=False,
                                skip_group_check=True,
                            )
                    z = z_grp[grp]
                    off = (t % CH2) * BL
                    for g in range(4):
                        nc.tensor.matmul(
                            z[:, g * 512 + off : g * 512 + off + BL],
                            u_sb[:, g * H : (g + 1) * H],
                            prev_h[grp],
                            start=False,
                            stop=True,
                            skip_group_check=True,
                        )

                def stage_gates(grp):
                    z = z_grp[grp]
                    off = (t % CH2) * BL
                    gt = g_cur[grp]
                    zs = z.rearrange("p (g n) -> p g n", g=4)[:, :, off : off + BL]
                    nc.scalar.activation(
                        gt[:, 0:64].rearrange("p (g n) -> p g n", g=4), zs, AF.Tanh
                    )

                def stage_dve(grp):
                    gt = g_cur[grp]
                    g_next = gp.tile([H, 96], f32, tag=f"g{grp}", name=f"gn{grp}")
                    pq = tp.tile([H, 2 * BL], f32, tag=f"pq{grp}")
                    nc.vector._custom_dve(
                        fma, out=pq[:, 0:BL], in0=gt[:, 0:BL], in1=gt[:, 32:48], s0=0.5
                    )
                    nc.vector._custom_dve(
                        fma, out=pq[:, BL:2*BL], in0=gt[:, BL:32], in1=gt[:, 64:80], s0=0.5
                    )
                    nc.vector.tensor_add(g_next[:, 64:80], pq[:, 0:BL], pq[:, BL:2*BL])
                    g_nxt[grp] = g_next

                def stage_tanhc(grp):
                    nc.scalar.activation(
                        g_cur[grp][:, 80:96], g_nxt[grp][:, 64:80], AF.Tanh
                    )

                def stage_h(grp):
                    gt = g_cur[grp]
                    hoff = (t % CH) * 32 + grp * BL
                    nc.vector._custom_dve(
                        fma,
                        out=hist_sb[:, hoff : hoff + BL],
                        in0=gt[:, 48:64],
                        in1=gt[:, 80:96],
                        s0=0.5,
                    )
                    prev_h[grp] = hist_sb[:, hoff : hoff + BL]
                    g_cur[grp] = g_nxt[grp]

                if ORDER == 1:
                    for grp in range(2):
                        stage_mm(grp)
                        stage_gates(grp)
                        stage_dve(grp)
                        stage_tanhc(grp)
                        stage_h(grp)
                else:
                    stage_mm(0)
                    stage_mm(1)
                    stage_gates(0)
                    stage_dve(0)
                    stage_gates(1)
                    stage_tanhc(0)
                    stage_dve(1)
                    stage_h(0)
                    stage_tanhc(1)
                    stage_h(1)
            nc.sync.dma_start(hist_d[(total // CH - 1) % nch], hist_sb[:])

    if not nc.is_finalized():
        nc.finalize()
    return nc



def _make_runner(T, repeat=1):
    """Build the Bass program and a cached jitted SPMD executor for it.

    Mirrors concourse.bass2jax.run_bass_via_pjrt, but keeps the jax.jit
    callable so repeated executions don't re-trace/re-compile.
    """
    import jax
    import concourse.mybir as mybir
    from concourse import bass2jax
    from jax.experimental.shard_map import shard_map
    from jax.sharding import Mesh, PartitionSpec

    bass2jax.install_neuronx_cc_hook()
    if KMODE == "v5":
        nc = _build_nc_v5(T, repeat)
    elif KMODE == "v4":
        nc = _build_nc_v4(T, repeat)
    elif KMODE == "v3":
        nc = _build_nc_v3(T, repeat)
    elif KMODE == "v2":
        nc = _build_nc_v2(T, repeat)
    else:
        nc = _build_nc(T, repeat)

    partition_name = nc.partition_id_tensor.name if nc.partition_id_tensor else None
    in_names, out_names, out_avals, zero_outs = [], [], [], []
    for alloc in nc.m.functions[0].allocations:
        if not isinstance(alloc, mybir.MemoryLocationSet):
            continue
        name = alloc.memorylocations[0].name
        if alloc.kind == "ExternalInput":
            if name != partition_name:
                in_names.append(name)
        elif alloc.kind == "ExternalOutput":
            out_names.append(name)
            shape = tuple(alloc.tensor_shape)
            dtype = mybir.dt.np(alloc.dtype)
            out_avals.append(jax.core.ShapedArray(shape, dtype))
            zero_outs.append(np.zeros(shape, dtype))
    n_params = len(in_names)
    n_outs = len(out_avals)
    all_in_names = list(in_names) + list(out_names)
    if partition_name is not None:
        all_in_names.append(partition_name)
    donate = tuple(range(n_params, n_params + n_outs))

    def _body(*args):
        operands = list(args)
        if partition_name is not None:
            operands.append(bass2jax.partition_id_tensor())
        outs = bass2jax._bass_exec_p.bind(
            *operands,
            out_avals=tuple(out_avals),
            in_names=tuple(all_in_names),
            out_names=tuple(out_names),
            lowering_input_output_aliases=(),
            sim_require_finite=True,
            sim_require_nnan=True,
            nc=nc,
        )
        return tuple(outs)

    devices = jax.devices()[:NCORES]
    mesh = Mesh(np.asarray(devices), ("core",))
    in_specs = (PartitionSpec("core"),) * (n_params + n_outs)
    out_specs = (PartitionSpec("core"),) * n_outs
    sharded = jax.jit(
        shard_map(_body, mesh=mesh, in_specs=in_specs, out_specs=out_specs,
                  check_rep=False),
        donate_argnums=donate,
        keep_unused=True,
    )

    from jax.sharding import NamedSharding

    sharding = NamedSharding(mesh, PartitionSpec("core"))

    def prepare(in_maps):
        per_core = [[np.asarray(m[name]) for name in in_names] for m in in_maps]
        concat_in = [
            np.concatenate([per_core[c][i] for c in range(NCORES)], axis=0)
            for i in range(n_params)
        ]
        return [jax.device_put(a, sharding) for a in concat_in]

    def fresh_zeros():
        return [
            jax.device_put(
                np.zeros((NCORES * z.shape[0], *z.shape[1:]), z.dtype), sharding
            )
            for z in zero_outs
        ]

    def execute(dev_in, dev_zeros):
        return sharded(*dev_in, *dev_zeros)

    def run(in_maps):
        out_arrs = execute(prepare(in_maps), fresh_zeros())
        return [
            {
                name: np.asarray(out_arrs[i]).reshape(NCORES, *out_avals[i].shape)[c]
                for i, name in enumerate(out_names)
            }
            for c in range(NCORES)
        ]

    run.prepare = prepare
    run.fresh_zeros = fresh_zeros
    run.execute = execute
    return run


def _get_runner(T, repeat=1):
    key = (T, repeat)
    if key not in _NC_CACHE:
        _NC_CACHE[key] = _make_runner(T, repeat)
    return _NC_CACHE[key]


def kernel(inputs, seq_lens, W, U, b, Wf, bf, _want_results=False, _trace=False):
    x = np.asarray(inputs, dtype=np.float32)
    seq_lens = np.asarray(seq_lens, dtype=np.int32)
    W = np.asarray(W, dtype=np.float32)
    U = np.asarray(U, dtype=np.float32)
    b = np.asarray(b, dtype=np.float32)
    Wf = np.asarray(Wf, dtype=np.float32)
    bf = np.asarray(bf, dtype=np.float32)

    B, T, Fdim = x.shape
    assert Fdim == F and B % NCORES == 0
    bl = B // NCORES
    assert bl == 32, "kernel is specialized to 32 samples/core"

    wt = np.concatenate([W, b[None, :]], axis=0)
    if KMODE in ("v2", "v3", "v4", "v5"):
        # one-func-tanh: gates stay in [i,f,g,o] order; i/f/o columns are
        # pre-scaled by 0.5 so sigma(z) = (tanh(z/2)+1)/2 uses only tanh
        scale = np.ones((4 * H,), np.float32)
        scale[0:2 * H] = 0.5
        scale[3 * H : 4 * H] = 0.5
        u16 = np.ascontiguousarray(U * scale).astype(np.float16)
        wt16 = np.ascontiguousarray(wt * scale).astype(np.float16)
    else:
        # v1: gate blocks permuted to [i, f, o, g] for the 3-bank sigmoid
        u16 = np.ascontiguousarray(U[:, GATE_PERM]).astype(np.float16)
        wt16 = np.ascontiguousarray(wt[:, GATE_PERM]).astype(np.float16)

    # x transposed per core: xt[f, t, j] = x[core*32 + j, t, f]; row F is 1.0
    in_maps = []
    for k in range(NCORES):
        xs = x[k * bl : (k + 1) * bl]  # [32, T, F]
        xt = np.empty((F + 1, T, bl), dtype=np.float16)
        xt[:F] = xs.transpose(2, 1, 0)
        xt[F] = 1.0
        in_maps.append({"xt": xt, "u": u16, "wt": wt16})

    run = _get_runner(T)
    results = run(in_maps)

    # host readout: h at t = seq_len - 1, then dense layer
    out = np.empty((B,), dtype=np.float32)
    wf = Wf[:, 0]
    for k in range(NCORES):
        hist = results[k]["hist"]  # [nch, H, CH*32] f16
        for j in range(bl):
            t = int(seq_lens[k * bl + j]) - 1
            h = hist[t // CH, :, (t % CH) * 32 + j].astype(np.float32)
            out[k * bl + j] = h @ wf + bf[0]
    if _want_results:
        return out, (run, in_maps)
    return out


if __name__ == "__main__":
    T = int(os.environ.get("T_STEPS", "128"))
    rng = np.random.default_rng(0)
    B = 256
    x = rng.standard_normal((B, T, F), dtype=np.float32)
    seq_lens = rng.integers(1, T + 1, size=(B,)).astype(np.int32)
    W = rng.standard_normal((F, 4 * H), dtype=np.float32) / np.sqrt(F)
    U = rng.standard_normal((H, 4 * H), dtype=np.float32) / np.sqrt(H)
    b = np.zeros((4 * H,), dtype=np.float32)
    Wf = rng.standard_normal((H, 1), dtype=np.float32) / np.sqrt(H)
    bf = np.zeros((1,), dtype=np.float32)

    def sig(v):
        return 1.0 / (1.0 + np.exp(-v))

    h = np.zeros((B, H), dtype=np.float32)
    cst = np.zeros((B, H), dtype=np.float32)
    for t in range(T):
        z = x[:, t] @ W + h @ U + b
        i, f, g, o = np.split(z, 4, axis=-1)
        i, f, g, o = sig(i), sig(f), np.tanh(g), sig(o)
        c_new = f * cst + i * g
        h_new = o * np.tanh(c_new)
        m = (t < seq_lens)[:, None]
        h = np.where(m, h_new, h)
        cst = np.where(m, c_new, cst)
    expected = (h @ Wf + bf).reshape(B)

    import time

    t0 = time.time()
    actual = kernel(x, seq_lens, W, U, b, Wf, bf)
    print(f"kernel() wall time: {time.time() - t0:.1f}s")
    err = np.linalg.norm(actual - expected) / np.linalg.norm(expected)
    print(f"Relative error: {err:.3e}")
    print("expected[:8]:", expected[:8])
    print("actual[:8]:  ", actual[:8])

